# revision 9
# baseline (speedup 1.0000x reference)
"""Trainium2 Bass kernel for nn_LSH: ret[o] = sum_{s,a} x[s] * w[o,s,a].

x: [1, 4096] f32, weights: [512, 4096, 128] f32 -> ret: [512] f32.

Sharding: out_dim 512 is split 64-per-core across 8 cores; x is replicated.
Per core the 64x4096x128 f32 slice (128 MiB) is streamed from HBM as a flat
[128, 262144] layout (partition p = o=p//2, s in [(p%2)*2048, ...+2048)).
Compute per chunk: DVE segmented reduce over the innermost a=128 giving
T[p, s_local]; partial x-multiply+reduce stages overlap the stream; a tiny
pairing matmul folds partition pairs (2o, 2o+1) into ret[o].
The tail chunks taper down so the last DVE reduce is short.
"""

import sys

sys.path.insert(0, "/opt/trn_rl_repo")

import numpy as np

import concourse.bass as bass
import concourse.mybir as mybir
import concourse.tile as tile
from concourse import bacc
from concourse.bass_utils import run_bass_kernel_spmd

P = 128
O_PER_CORE = 64
N_CORES = 8
S = 4096
A = 128
COLS = O_PER_CORE * S * A // P  # 262144 per-partition row length
SLOC = 2048  # s-values covered by each partition

# Chunk schedule: full 4 MiB DMAs for max bandwidth; the tail tapers so the
# final DVE reduce after the last landing is short. bufs=5 keeps the HWDGE
# ring backed up ~4 chunks deep so dispatch is never gated by the DVE.
CHUNKS = [8192] * 31 + [4096, 2048, 1024, 1024]
assert sum(CHUNKS) == COLS
# After these chunk indices, run a fused partial x-multiply+reduce stage.
PARTIAL_AFTER = [7, 15, 23, 30, 33, 34]
NPART = len(PARTIAL_AFTER)

_CACHED_NC = None


def _build_nc():
    nc = bacc.Bacc(
        "TRN2",
        target_bir_lowering=False,
        debug=False,
        num_devices=N_CORES,
    )
    # wmain: 31 full 4-MiB chunks, each chunk a contiguous [128, 8192] block
    # (row-major, 32 KB per partition row) so every SDMA engine reads
    # sequential DRAM addresses. wtail: last 8192 cols in [P, cols] layout.
    wmain = nc.dram_tensor(
        "wmain", [31 * P, 8192], mybir.dt.float32, kind="ExternalInput"
    ).ap()
    wtail = nc.dram_tensor(
        "wtail", [P, 8192], mybir.dt.float32, kind="ExternalInput"
    ).ap()
    xt = nc.dram_tensor("xt", [P, SLOC], mybir.dt.float32, kind="ExternalInput").ap()
    pmat = nc.dram_tensor(
        "pmat", [P, O_PER_CORE], mybir.dt.float32, kind="ExternalInput"
    ).ap()
    out = nc.dram_tensor(
        "out", [O_PER_CORE, 1], mybir.dt.float32, kind="ExternalOutput"
    ).ap()

    with tile.TileContext(nc) as tc:
        with (
            tc.tile_pool(name="wp", bufs=5) as wp,
            tc.tile_pool(name="const", bufs=1) as constp,
            tc.tile_pool(name="accp", bufs=1) as accp,
            tc.tile_pool(name="psum", bufs=1, space="PSUM") as psp,
        ):
            acc = accp.tile([P, SLOC], mybir.dt.float32)
            accx = accp.tile([P, SLOC], mybir.dt.float32)
            vparts = accp.tile([P, NPART], mybir.dt.float32)
            xt_t = constp.tile([P, SLOC], mybir.dt.float32)
            pm_t = constp.tile([P, O_PER_CORE], mybir.dt.float32)

            coff = 0  # acc column offset (completed s-values)
            boundaries = []  # acc col ranges per partial stage
            pstart = 0
            pi = 0
            tail0 = 31 * 8192  # col offset where the tail region starts
            for k, cols in enumerate(CHUNKS):
                wt = wp.tile([P, max(CHUNKS)], mybir.dt.float32, tag="wt")
                nseg = cols // A
                if k < 31:
                    nc.sync.dma_start(wt[:, :cols], wmain[k * P : (k + 1) * P, :])
                else:
                    toff = coff * A - tail0
                    nc.sync.dma_start(wt[:, :cols], wtail[:, toff : toff + cols])
                if k == 1:
                    # Constants go via SWDGE so the HWDGE queue carries
                    # only the weight stream.
                    nc.gpsimd.dma_start(xt_t[:], xt[:])
                    nc.gpsimd.dma_start(pm_t[:], pmat[:])
                seg = wt[:, :cols].rearrange("p (n a) -> p n a", a=A)
                nc.vector.tensor_reduce(
                    acc[:, coff : coff + nseg],
                    seg,
                    axis=mybir.AxisListType.X,
                    op=mybir.AluOpType.add,
                )
                coff += nseg
                if k == PARTIAL_AFTER[pi]:
                    nc.vector.tensor_mul(
                        accx[:, pstart:coff], acc[:, pstart:coff], xt_t[:, pstart:coff]
                    )
                    nc.vector.tensor_reduce(
                        vparts[:, pi : pi + 1],
                        accx[:, pstart:coff],
                        axis=mybir.AxisListType.X,
                        op=mybir.AluOpType.add,
                    )
                    boundaries.append((pstart, coff))
                    pstart = coff
                    pi += 1
            assert coff == SLOC and pi == NPART

            v = accp.tile([P, 1], mybir.dt.float32)
            nc.vector.tensor_reduce(
                v[:], vparts[:], axis=mybir.AxisListType.X, op=mybir.AluOpType.add
            )
            ps = psp.tile([O_PER_CORE, 1], mybir.dt.float32)
            nc.tensor.matmul(ps[:], pm_t[:], v[:], start=True, stop=True)
            res = accp.tile([O_PER_CORE, 1], mybir.dt.float32)
            nc.scalar.copy(res[:], ps[:])
            nc.sync.dma_start(out[:], res[:])

    nc.compile()
    return nc


def _get_nc():
    global _CACHED_NC
    if _CACHED_NC is None:
        _CACHED_NC = _build_nc()
    return _CACHED_NC


def _in_maps(x, weights):
    x = np.ascontiguousarray(np.asarray(x, dtype=np.float32))
    weights = np.asarray(weights, dtype=np.float32)
    xt = np.tile(x.reshape(2, SLOC), (P // 2, 1))
    pmat = np.zeros((P, O_PER_CORE), dtype=np.float32)
    pmat[np.arange(P), np.arange(P) // 2] = 1.0
    maps = []
    ntail = 8192
    for c in range(N_CORES):
        wc = np.ascontiguousarray(
            weights[c * O_PER_CORE : (c + 1) * O_PER_CORE]
        ).reshape(P, COLS)
        wmain = np.ascontiguousarray(
            wc[:, : COLS - ntail].reshape(P, 31, 8192).transpose(1, 0, 2)
        ).reshape(31 * P, 8192)
        wtail = np.ascontiguousarray(wc[:, COLS - ntail :])
        maps.append({"wmain": wmain, "wtail": wtail, "xt": xt, "pmat": pmat})
    return maps


def run(x, weights, trace=False):
    """Run on hardware; returns (ret[512], BassKernelResults)."""
    nc = _get_nc()
    res = run_bass_kernel_spmd(
        nc, _in_maps(x, weights), list(range(N_CORES)), trace=trace
    )
    ret = np.concatenate(
        [res.results[c]["out"].reshape(O_PER_CORE) for c in range(N_CORES)]
    ).astype(np.float32)
    return ret, res


def kernel(x, weights):
    ret, _ = run(x, weights)
    return ret



# revision 12
# speedup vs baseline: 1.8253x; 1.8253x over previous
"""Trainium2 Bass kernel for nn_LSH: ret[o] = sum_{s,a} x[s] * w[o,s,a].

x: [1, 4096] f32, weights: [512, 4096, 128] f32 -> ret: [512] f32.

Sharding: out_dim 512 is split 64-per-core across 8 cores; x is replicated.
The kernel is HBM-bandwidth-bound, so weights are uploaded as bf16 (host-side
cast, 64 MiB per core instead of 128 MiB) — the 2e-2 relative-error budget
admits bf16 with ~4x margin (measured ~6e-3).

Per core the 64x4096x128 bf16 slice streams as 19 chunks; full chunks are
4 MiB, stored chunk-contiguous in DRAM so every SDMA engine reads sequential
addresses. Partition p holds (o = p//2, s-half = p%2): 2048 s-values x 128 a.
Compute per chunk: a binary-tree pairwise add over the innermost a=128 on the
DVE (tensor_add runs in 2x packed mode for bf16, unlike the 1x-only
tensor_reduce), 7 passes 128->1, writing T[p, s] into acc. Partial
x-multiply+reduce stages overlap the stream; a tiny pairing matmul folds
partition pairs (2o, 2o+1) into ret[o]. The tail chunks taper down so the
last tree after the final landing is short.
"""

import sys

sys.path.insert(0, "/opt/trn_rl_repo")

import ml_dtypes
import numpy as np

import concourse.bass as bass
import concourse.mybir as mybir
import concourse.tile as tile
from concourse import bacc
from concourse.bass_utils import run_bass_kernel_spmd

BF16 = ml_dtypes.bfloat16

P = 128
O_PER_CORE = 64
N_CORES = 8
S = 4096
A = 128
SLOC = 2048  # s-values covered by each partition
COLS = SLOC * A  # 262144 bf16 elems per partition row

NFULL = 15  # full 4-MiB chunks (16384 elems/partition each)
FULL = 16384
# Chunk schedule in elems/partition; tail tapers so the final tree is short.
CHUNKS = [FULL] * NFULL + [8192, 4096, 2048, 2048]
assert sum(CHUNKS) == COLS
# After these chunk indices, run a partial x-multiply+reduce stage.
PARTIAL_AFTER = [2, 5, 8, 11, 14, 18]
NPART = len(PARTIAL_AFTER)

_CACHED_NC = None


def _build_nc():
    nc = bacc.Bacc(
        "TRN2",
        target_bir_lowering=False,
        debug=False,
        num_devices=N_CORES,
    )
    wmain = nc.dram_tensor(
        "wmain", [NFULL * P, FULL], mybir.dt.bfloat16, kind="ExternalInput"
    ).ap()
    wtail = nc.dram_tensor(
        "wtail", [P, FULL], mybir.dt.bfloat16, kind="ExternalInput"
    ).ap()
    xt = nc.dram_tensor("xt", [P, SLOC], mybir.dt.bfloat16, kind="ExternalInput").ap()
    pmat = nc.dram_tensor(
        "pmat", [P, O_PER_CORE], mybir.dt.float32, kind="ExternalInput"
    ).ap()
    out = nc.dram_tensor(
        "out", [O_PER_CORE, 1], mybir.dt.float32, kind="ExternalOutput"
    ).ap()

    with tile.TileContext(nc) as tc:
        with (
            tc.tile_pool(name="wp", bufs=4) as wp,
            tc.tile_pool(name="scp", bufs=1) as scp,
            tc.tile_pool(name="const", bufs=1) as constp,
            tc.tile_pool(name="accp", bufs=1) as accp,
            tc.tile_pool(name="psum", bufs=1, space="PSUM") as psp,
        ):
            acc = accp.tile([P, SLOC], mybir.dt.bfloat16)
            accx = accp.tile([P, SLOC], mybir.dt.bfloat16)
            vparts = accp.tile([P, NPART], mybir.dt.float32)
            xt_t = constp.tile([P, SLOC], mybir.dt.bfloat16)
            pm_t = constp.tile([P, O_PER_CORE], mybir.dt.float32)

            tail0 = NFULL * FULL
            soff = 0  # acc column offset (completed s-values)
            pstart = 0
            pi = 0
            for k, elems in enumerate(CHUNKS):
                n = elems // A  # s-values in this chunk
                wt = wp.tile([P, FULL], mybir.dt.bfloat16, tag="wt")
                if k < NFULL:
                    nc.sync.dma_start(wt[:, :elems], wmain[k * P : (k + 1) * P, :])
                else:
                    toff = soff * A - tail0
                    nc.sync.dma_start(wt[:, :elems], wtail[:, toff : toff + elems])
                if k == 1:
                    # Constants go via SWDGE so the HWDGE queue carries
                    # only the weight stream.
                    nc.gpsimd.dma_start(xt_t[:], xt[:])
                    nc.gpsimd.dma_start(pm_t[:], pmat[:])

                # Binary-tree reduce over a: 128 -> 64 -> ... -> 1.
                # Scratch regions for widths 64..2 laid out back to back.
                # widths 64+32+16+8+4+2 = 126 per s-value, laid back to back
                sc = scp.tile([P, 126 * (FULL // A)], mybir.dt.bfloat16, tag="sc")
                src = wt[:, :elems].rearrange("p (n a) -> p n a", a=A)
                off = 0
                w_ = A
                while w_ > 2:
                    w_ //= 2
                    dst = sc[:, off : off + n * w_].rearrange(
                        "p (n a) -> p n a", a=w_
                    )
                    nc.vector.tensor_add(dst, src[:, :, :w_], src[:, :, w_:])
                    src = dst
                    off += n * w_
                # Last pass: width 2 -> 1 into acc (2B-aligned in1: 1x mode).
                dst = acc[:, soff : soff + n].rearrange("p (n a) -> p n a", a=1)
                nc.vector.tensor_add(dst, src[:, :, 0:1], src[:, :, 1:2])

                soff += n
                if k == PARTIAL_AFTER[pi]:
                    nc.vector.tensor_mul(
                        accx[:, pstart:soff], acc[:, pstart:soff], xt_t[:, pstart:soff]
                    )
                    nc.vector.tensor_reduce(
                        vparts[:, pi : pi + 1],
                        accx[:, pstart:soff],
                        axis=mybir.AxisListType.X,
                        op=mybir.AluOpType.add,
                    )
                    pstart = soff
                    pi += 1
            assert soff == SLOC and pi == NPART

            v = accp.tile([P, 1], mybir.dt.float32)
            nc.vector.tensor_reduce(
                v[:], vparts[:], axis=mybir.AxisListType.X, op=mybir.AluOpType.add
            )
            ps = psp.tile([O_PER_CORE, 1], mybir.dt.float32)
            nc.tensor.matmul(ps[:], pm_t[:], v[:], start=True, stop=True)
            res = accp.tile([O_PER_CORE, 1], mybir.dt.float32)
            nc.scalar.copy(res[:], ps[:])
            nc.sync.dma_start(out[:], res[:])

    nc.compile()
    return nc


def _get_nc():
    global _CACHED_NC
    if _CACHED_NC is None:
        _CACHED_NC = _build_nc()
    return _CACHED_NC


def _in_maps(x, weights):
    x = np.ascontiguousarray(np.asarray(x, dtype=np.float32))
    weights = np.asarray(weights, dtype=np.float32)
    xt = np.tile(x.reshape(2, SLOC).astype(BF16), (P // 2, 1))
    pmat = np.zeros((P, O_PER_CORE), dtype=np.float32)
    pmat[np.arange(P), np.arange(P) // 2] = 1.0
    maps = []
    tail0 = NFULL * FULL
    for c in range(N_CORES):
        wc = (
            weights[c * O_PER_CORE : (c + 1) * O_PER_CORE]
            .astype(BF16)
            .reshape(P, COLS)
        )
        wmain = np.ascontiguousarray(
            wc[:, :tail0].reshape(P, NFULL, FULL).transpose(1, 0, 2)
        ).reshape(NFULL * P, FULL)
        wtail = np.ascontiguousarray(wc[:, tail0:])
        maps.append({"wmain": wmain, "wtail": wtail, "xt": xt, "pmat": pmat})
    return maps


def run(x, weights, trace=False):
    """Run on hardware; returns (ret[512], BassKernelResults)."""
    nc = _get_nc()
    res = run_bass_kernel_spmd(
        nc, _in_maps(x, weights), list(range(N_CORES)), trace=trace
    )
    ret = np.concatenate(
        [res.results[c]["out"].reshape(O_PER_CORE) for c in range(N_CORES)]
    ).astype(np.float32)
    return ret, res


def kernel(x, weights):
    ret, _ = run(x, weights)
    return ret
